# revision 1
# baseline (speedup 1.0000x reference)
"""Multi-head self-attention (B=8, T=2048, C=192, H=6, HS=32) on 8 TRN2 cores.

Sharding: data-parallel over batch — core i computes batch element i fully
on-chip (no collectives). Host pre-transposes x and packs weights so the
device does zero transposes:

  qT/kT [d, t] = Wq_packed.T @ x.T          (d = h*HS + dd)
  v     [s, d] = x @ Wv_packed, stored per-head as [v_h | ones] (33 cols)
  S^T   [s, t] = kT_h.T @ qT_h              (K=32 matmuls, row-group packed)
  P^T          = exp(S^T / sqrt(HS))        (ScalarE, PSUM->SBUF, bf16)
  [O^T_h; rowsum_h x32] = [v_h|1x32].T @ P^T  (rowsum replicated to rows 32-63)
  OTn_h [d, t] = O^T_h * (1/rowsum_h)       (DVE reciprocal + mul, no bcast)
  out   [t, c] = sum_h OTn_h.T @ Wproj_h + bias  (K=32 accum + rank-1 bias)
"""

import numpy as np
import ml_dtypes
from contextlib import ExitStack

import concourse.bass as bass
import concourse.tile as tile
from concourse import bacc, mybir
from concourse.bass_utils import run_bass_kernel_spmd

B, T, C = 8, 2048, 192
H, HS = 6, 32
P = 128
TCH = 512            # t-chunk width (one PSUM bank of fp32)
NT = T // TCH        # 4
NS = T // P          # 16 s-tiles
SCALE = 1.0 / float(np.sqrt(HS))
BF16 = mybir.dt.bfloat16
F32 = mybir.dt.float32
Exp = mybir.ActivationFunctionType.Exp

_CACHE = {}


def build_nc():
    nc = bacc.Bacc()
    xT = nc.declare_dram_parameter("xT", [C, T], BF16, isOutput=False)
    wq = nc.declare_dram_parameter("wq", [C, H * HS], BF16, isOutput=False)
    wk = nc.declare_dram_parameter("wk", [C, H * HS], BF16, isOutput=False)
    wv = nc.declare_dram_parameter("wv", [C, H * HS], BF16, isOutput=False)
    wp = nc.declare_dram_parameter("wp", [H, HS, C], BF16, isOutput=False)
    bp = nc.declare_dram_parameter("bp", [1, C], BF16, isOutput=False)
    out = nc.declare_dram_parameter("out", [T, C], F32, isOutput=True)

    with tile.TileContext(nc) as tc, ExitStack() as ctx:
        singles = ctx.enter_context(tc.tile_pool(name="singles", bufs=1))
        qk_pool = ctx.enter_context(tc.tile_pool(name="qk", bufs=1))
        vaug_pool = ctx.enter_context(tc.tile_pool(name="vaug", bufs=1))
        pt_pool = ctx.enter_context(tc.tile_pool(name="ptp", bufs=4))
        otn_pool = ctx.enter_context(tc.tile_pool(name="otn", bufs=1))
        small = ctx.enter_context(tc.tile_pool(name="small", bufs=4))
        ysb_pool = ctx.enter_context(tc.tile_pool(name="ysb", bufs=3))

        # ---------------- load inputs ----------------
        xT_a = singles.tile([P, T], BF16)
        nc.sync.dma_start(xT_a, xT[0:P, :])
        xT_b = singles.tile([C - P, T], BF16)
        nc.sync.dma_start(xT_b, xT[P:C, :])

        w_sb = {}
        for name, dram in (("q", wq), ("k", wk), ("v", wv)):
            a = singles.tile([P, H * HS], BF16, name=f"w{name}a")
            nc.sync.dma_start(a, dram[0:P, :])
            b = singles.tile([C - P, H * HS], BF16, name=f"w{name}b")
            nc.sync.dma_start(b, dram[P:C, :])
            w_sb[name] = (a, b)

        wp_sb = []
        for h in range(H):
            wph = singles.tile([HS, C], BF16, name=f"wp{h}")
            nc.sync.dma_start(wph, wp[h, :, :])
            wp_sb.append(wph)
        bp_sb = singles.tile([1, C], BF16)
        nc.sync.dma_start(bp_sb, bp[:, :])
        ones1 = singles.tile([1, P], BF16)
        nc.vector.memset(ones1, 1.0)

        # ---------------- phase 1: qT, kT, v_aug ----------------
        qT_a = qk_pool.tile([P, T], BF16)       # heads 0..3, d-major
        qT_b = qk_pool.tile([C - P, T], BF16)   # heads 4,5
        kT_a = qk_pool.tile([P, T], BF16)
        kT_b = qk_pool.tile([C - P, T], BF16)
        v_aug = []
        with tc.tile_pool(name="pqkv", bufs=2, space="PSUM") as pqkv:
            for proj, dst_a, dst_b in (("q", qT_a, qT_b), ("k", kT_a, kT_b)):
                wa, wb = w_sb[proj]
                for dlo, dsz, dst in ((0, P, dst_a), (P, C - P, dst_b)):
                    for t0 in range(0, T, TCH):
                        ps = pqkv.tile([P, TCH], F32, name="psq", tag="psq")
                        nc.tensor.matmul(
                            ps[0:dsz, :], wa[:, dlo:dlo + dsz],
                            xT_a[:, t0:t0 + TCH], start=True, stop=False)
                        nc.tensor.matmul(
                            ps[0:dsz, :], wb[:, dlo:dlo + dsz],
                            xT_b[:, t0:t0 + TCH], start=False, stop=True)
                        nc.vector.tensor_copy(
                            dst[0:dsz, t0:t0 + TCH], ps[0:dsz, :])
            wva, wvb = w_sb["v"]
            for si in range(NS):
                s0 = si * P
                va = vaug_pool.tile(
                    [P, H * 2 * HS], BF16, name=f"vaug{si}", tag=f"vaug{si}")
                ps = pqkv.tile([P, H * HS], F32, name="psv", tag="psv")
                nc.tensor.matmul(ps, xT_a[:, s0:s0 + P], wva,
                                 start=True, stop=False)
                nc.tensor.matmul(ps, xT_b[:, s0:s0 + P], wvb,
                                 start=False, stop=True)
                va_r = va.rearrange("p (h e) -> p h e", h=H)
                ps_r = ps.rearrange("p (h d) -> p h d", h=H)
                nc.vector.tensor_copy(va_r[:, :, 0:HS], ps_r)
                nc.vector.memset(va_r[:, :, HS:2 * HS], 1.0)
                v_aug.append(va)

        # ---------------- phase 2: attention ----------------
        otn = [otn_pool.tile([HS, T], BF16, name=f"otn{h}", tag=f"otn{h}")
               for h in range(H)]
        # head pairs (A=2p, B=2p+1); within a pair kT/qT rows sit in
        # distinct 32-row groups, so the two QKT matmuls run concurrently
        def hsrc(h):
            if h < 4:
                return kT_a, qT_a, HS * h
            return kT_b, qT_b, HS * (h - 4)
        with (
            tc.tile_pool(name="pst", bufs=2, space="PSUM") as pst_pool,
            tc.tile_pool(name="pav", bufs=1, space="PSUM") as pav_pool,
            tc.tile_pool(name="py", bufs=1, space="PSUM") as py_pool,
        ):
            for tc0 in range(0, T, TCH):
                av = [pav_pool.tile([P, TCH], F32,
                                    name=f"avp{p}", tag=f"avp{p}")
                      for p in range(H // 2)]
                for si in range(NS):
                    s0 = si * P
                    for p in range(H // 2):
                        hA, hB = 2 * p, 2 * p + 1
                        stp = pst_pool.tile([P, 2 * TCH], F32,
                                            name="stp", tag="stp")
                        for half, h in ((0, hA), (1, hB)):
                            kT_t, qT_t, pb = hsrc(h)
                            nc.tensor.matmul(
                                stp[:, half * TCH:(half + 1) * TCH],
                                kT_t[pb:pb + HS, s0:s0 + P],
                                qT_t[pb:pb + HS, tc0:tc0 + TCH],
                                start=True, stop=True, tile_position=(pb, 0))
                        ptp = pt_pool.tile([P, 2 * TCH], BF16,
                                           name="ptp", tag="ptp")
                        nc.scalar.activation(ptp, stp, Exp, scale=SCALE)
                        for half, h in ((0, hA), (1, hB)):
                            nc.tensor.matmul(
                                av[p][64 * half:64 * half + 64, :],
                                v_aug[si][:, 2 * HS * h:2 * HS * (h + 1)],
                                ptp[:, half * TCH:(half + 1) * TCH],
                                start=(si == 0), stop=(si == NS - 1),
                                skip_group_check=True,
                                tile_position=(0, 64 * half))
                for p in range(H // 2):
                    rbp = small.tile([P, TCH], F32, name="rbp", tag="rbp")
                    for half, h in ((0, 2 * p), (1, 2 * p + 1)):
                        b = 64 * half
                        nc.vector.reciprocal(
                            rbp[b:b + HS, :], av[p][b + HS:b + 2 * HS, :])
                        nc.vector.tensor_mul(
                            otn[h][:, tc0:tc0 + TCH],
                            av[p][b:b + HS, :], rbp[b:b + HS, :])
                # ---- projection for this t-chunk (spare PSUM bank) ----
                for tt in range(tc0, tc0 + TCH, P):
                    ps = py_pool.tile([P, C], F32, name="psy", tag="psy")
                    nc.tensor.matmul(ps, ones1, bp_sb, start=True, stop=False)
                    for h in range(H):
                        nc.tensor.matmul(
                            ps, otn[h][:, tt:tt + P], wp_sb[h],
                            start=False, stop=(h == H - 1))
                    ysb = ysb_pool.tile([P, C], F32, name="ysbt", tag="ysbt")
                    nc.vector.tensor_copy(ysb, ps)
                    nc.sync.dma_start(out[tt:tt + P, :], ysb)

    nc.compile()
    return nc


def _get_nc():
    if "nc" not in _CACHE:
        _CACHE["nc"] = build_nc()
    return _CACHE["nc"]


def make_in_maps(x, Wq, Wk, Wv, Wproj, bproj):
    bf = ml_dtypes.bfloat16
    x = np.asarray(x, np.float32)
    pack = lambda w: np.ascontiguousarray(
        np.transpose(np.asarray(w, np.float32), (1, 0, 2)).reshape(C, H * HS)
    ).astype(bf)
    wq, wk, wv = pack(Wq), pack(Wk), pack(Wv)
    wp = np.ascontiguousarray(
        np.asarray(Wproj, np.float32).reshape(H, HS, C)).astype(bf)
    bp = np.asarray(bproj, np.float32).reshape(1, C).astype(bf)
    maps = []
    for i in range(B):
        xti = np.ascontiguousarray(x[i].T).astype(bf)
        maps.append({"xT": xti, "wq": wq, "wk": wk, "wv": wv,
                     "wp": wp, "bp": bp})
    return maps


def run(inputs, trace=False, **kw):
    nc = _get_nc()
    in_maps = make_in_maps(**inputs)
    res = run_bass_kernel_spmd(nc, in_maps, core_ids=list(range(B)),
                               trace=trace, **kw)
    y = np.stack([np.asarray(res.results[i]["out"], np.float32)
                  for i in range(B)], axis=0)
    return y, res


def kernel(**inputs):
    y, _ = run(inputs, trace=False)
    return y



# revision 6
# speedup vs baseline: 1.2961x; 1.2961x over previous
"""Multi-head self-attention (B=8, T=2048, C=192, H=6, HS=32) on 8 TRN2 cores.

Sharding: data-parallel over batch — core i computes batch element i fully
on-chip (no collectives). Host pre-transposes x and packs weights so the
device does zero transposes.

Per-core pipeline (engine in parentheses):
  qT/kT [d, t] = W^T @ xT                   (PE; PSUM->SBUF copies on ACT)
  v_aug [s, (h|1)]                          (PE; DVE copies; ones col -> rowsum)
  S^T   [s, t] = kT_h^T @ qT_h              (PE, K=32, one f32 PSUM bank/head)
  P^T = exp(S/sqrt(HS)) per head tile:      exact exp on ACT, or Schraudolph
        bf16-bit affine on DVE (bits = trunc(S1*x + S2) as int16, bitcast
        to bf16)
  O[t, d|rowsum] += P^T_tile^T @ v_aug      (PE, free dim 33 per head)
  O' = O * recip(rowsum)                    (DVE recip + broadcast mul)
  O'^T via PE transpose (identity), DVE copies to SBUF
  y[t, c] = O'^T.T @ Wp + bias              (PE, K=192 in 2 chunks + bias row)
"""

import numpy as np
import ml_dtypes
from contextlib import ExitStack

import concourse.bass as bass
import concourse.tile as tile
from concourse import bacc, mybir
from concourse.bass_utils import run_bass_kernel_spmd

B, T, C = 8, 2048, 192
H, HS = 6, 32
P = 128
TCH = 512            # t-chunk width per head (pair tile = 2*TCH)
NT = T // TCH        # 4
NS = T // P          # 16 s-tiles
E1 = HS + 1          # 33: per-head AV free dim (32 d + rowsum)
SCALE = 1.0 / float(np.sqrt(HS))
BF16 = mybir.dt.bfloat16
F32 = mybir.dt.float32
I16 = mybir.dt.int16
Exp = mybir.ActivationFunctionType.Exp
Alu = mybir.AluOpType

# Schraudolph bf16-bits exp: bf16(trunc(S1*x + S2)) ~ exp(SCALE*x), |rel|<4%
S1 = float((128.0 / np.log(2.0)) * SCALE)
S2 = 16250.0

# exp engine per (si, head): 'a' = ACT exact exp, 'd' = DVE Schraudolph.
# Heads 0-2 exact on ACT (plus head 3 every 3rd si); heads 3-5 approx on DVE.
def exp_pat(si, h):
    if h < 3:
        return "a"
    if h == 3 and si % 3 == 0:
        return "a"
    return "d"

_CACHE = {}


def build_nc():
    nc = bacc.Bacc()
    xT = nc.declare_dram_parameter("xT", [C, T], BF16, isOutput=False)
    wq = nc.declare_dram_parameter("wq", [C, H * HS], BF16, isOutput=False)
    wk = nc.declare_dram_parameter("wk", [C, H * HS], BF16, isOutput=False)
    wv = nc.declare_dram_parameter("wv", [C, H * HS], BF16, isOutput=False)
    wp = nc.declare_dram_parameter("wp", [H * HS, C], BF16, isOutput=False)
    bp = nc.declare_dram_parameter("bp", [1, C], BF16, isOutput=False)
    ident = nc.declare_dram_parameter("ident", [P, P], BF16, isOutput=False)
    out = nc.declare_dram_parameter("out", [T, C], F32, isOutput=True)

    with tile.TileContext(nc) as tc, ExitStack() as ctx:
        singles = ctx.enter_context(tc.tile_pool(name="singles", bufs=1))
        qk_pool = ctx.enter_context(tc.tile_pool(name="qk", bufs=1))
        pt_pool = ctx.enter_context(tc.tile_pool(name="ptp", bufs=10))
        post_pool = ctx.enter_context(tc.tile_pool(name="post", bufs=2))
        ysb_pool = ctx.enter_context(tc.tile_pool(name="ysb", bufs=3))

        # ---------------- load inputs ----------------
        xT_a = singles.tile([P, T], BF16)
        nc.sync.dma_start(xT_a, xT[0:P, :])
        xT_b = singles.tile([C - P, T], BF16)
        nc.sync.dma_start(xT_b, xT[P:C, :])

        w_sb = {}
        for name, dram in (("q", wq), ("k", wk), ("v", wv)):
            a = singles.tile([P, H * HS], BF16, name=f"w{name}a")
            nc.sync.dma_start(a, dram[0:P, :])
            b = singles.tile([C - P, H * HS], BF16, name=f"w{name}b")
            nc.sync.dma_start(b, dram[P:C, :])
            w_sb[name] = (a, b)

        wp_a = singles.tile([P, C], BF16, name="wpa")
        nc.sync.dma_start(wp_a, wp[0:P, :])
        wp_b = singles.tile([H * HS - P, C], BF16, name="wpb")
        nc.sync.dma_start(wp_b, wp[P:H * HS, :])
        bp_sb = singles.tile([1, C], BF16)
        nc.sync.dma_start(bp_sb, bp[:, :])
        id_sb = singles.tile([P, P], BF16, name="idsb")
        nc.sync.dma_start(id_sb, ident[:, :])
        ones1 = singles.tile([1, P], BF16)
        nc.vector.memset(ones1, 1.0)

        # preload exp activation table off the critical path
        warm = singles.tile([1, P], BF16, name="warm")
        nc.scalar.activation(warm, ones1, Exp)

        # v_aug: [s, si*(h|1)] with ones in col 32 of each head group
        v_aug = singles.tile([P, NS * H * E1], BF16, name="vaug")
        nc.vector.memset(v_aug, 1.0)
        va_r = v_aug.rearrange("p (s h e) -> p s h e", s=NS, h=H)

        # ---------------- phase 1: qT, kT, v_aug ----------------
        qT_a = qk_pool.tile([P, T], BF16)       # heads 0..3, d-major
        qT_b = qk_pool.tile([C - P, T], BF16)   # heads 4,5
        kT_a = qk_pool.tile([P, T], BF16)
        kT_b = qk_pool.tile([C - P, T], BF16)
        with tc.tile_pool(name="pqkv", bufs=2, space="PSUM") as pqkv:
            for proj, dst_a, dst_b in (("q", qT_a, qT_b), ("k", kT_a, kT_b)):
                wa, wb = w_sb[proj]
                for dlo, dsz, dst in ((0, P, dst_a), (P, C - P, dst_b)):
                    for t0 in range(0, T, 2 * TCH):
                        ps = pqkv.tile([P, 2 * TCH], F32, name="psq", tag="psq")
                        for th in (0, TCH):
                            nc.tensor.matmul(
                                ps[0:dsz, th:th + TCH], wa[:, dlo:dlo + dsz],
                                xT_a[:, t0 + th:t0 + th + TCH],
                                start=True, stop=False)
                            nc.tensor.matmul(
                                ps[0:dsz, th:th + TCH], wb[:, dlo:dlo + dsz],
                                xT_b[:, t0 + th:t0 + th + TCH],
                                start=False, stop=True)
                        nc.scalar.copy(
                            dst[0:dsz, t0:t0 + 2 * TCH], ps[0:dsz, :])
            wva, wvb = w_sb["v"]
            for si in range(NS):
                s0 = si * P
                ps = pqkv.tile([P, H * HS], F32, name="psv", tag="psv")
                nc.tensor.matmul(ps, xT_a[:, s0:s0 + P], wva,
                                 start=True, stop=False)
                nc.tensor.matmul(ps, xT_b[:, s0:s0 + P], wvb,
                                 start=False, stop=True)
                ps_r = ps.rearrange("p (h d) -> p h d", h=H)
                nc.vector.tensor_copy(va_r[:, si, :, 0:HS], ps_r)

        # ---------------- phase 2: attention ----------------
        def hsrc(h):
            if h < 4:
                return kT_a, qT_a, HS * h
            return kT_b, qT_b, HS * (h - 4)

        with (
            tc.tile_pool(name="pst", bufs=4, space="PSUM") as pst_pool,
            tc.tile_pool(name="pav", bufs=1, space="PSUM") as pav_pool,
            tc.tile_pool(name="py", bufs=2, space="PSUM") as py_pool,
        ):
            def postprocess(tc0, av):
                """normalize, transpose, project, store one t-chunk."""
                for tt in range(NT):
                    av_t = av[tt // 2].rearrange(
                        "p (u h e) -> p u h e", u=2, h=H)
                    u = tt % 2
                    rec = post_pool.tile([P, H], F32, name="rec", tag="rec")
                    nc.vector.reciprocal(rec, av_t[:, u, :, HS])
                    onorm = post_pool.tile([P, H * HS], BF16,
                                           name="onorm", tag="onorm")
                    on_r = onorm.rearrange("p (h e) -> p h e", h=H)
                    nc.vector.tensor_tensor(
                        on_r, av_t[:, u, :, 0:HS],
                        rec.unsqueeze(2).to_broadcast([P, H, HS]),
                        Alu.mult)
                    # proj psum bank doubles as transpose scratch: cols
                    # [C, C+P) viewed as bf16 hold O'^T before DVE copy-out
                    ps = py_pool.tile([P, C + P], F32, name="psy", tag="psy")
                    tp = ps[:, C:C + P].bitcast(BF16)
                    nc.tensor.transpose(tp[:, 0:P], onorm[:, 0:P], id_sb)
                    nc.tensor.transpose(
                        tp[0:H * HS - P, P:2 * P], onorm[:, P:H * HS], id_sb)
                    oT1 = post_pool.tile([P, P], BF16, name="oT1", tag="oT1")
                    nc.vector.tensor_copy(oT1, tp[:, 0:P])
                    oT2 = post_pool.tile([H * HS - P, P], BF16,
                                         name="oT2", tag="oT2")
                    nc.vector.tensor_copy(oT2, tp[0:H * HS - P, P:2 * P])
                    nc.tensor.matmul(ps[:, 0:C], ones1, bp_sb,
                                     start=True, stop=False)
                    nc.tensor.matmul(ps[:, 0:C], oT1, wp_a,
                                     start=False, stop=False)
                    nc.tensor.matmul(ps[:, 0:C], oT2, wp_b,
                                     start=False, stop=True)
                    ysb = ysb_pool.tile([P, C], F32, name="ysbt", tag="ysbt")
                    nc.vector.tensor_copy(ysb, ps[:, 0:C])
                    nc.sync.dma_start(out[tc0 + tt * P:tc0 + (tt + 1) * P, :],
                                      ysb)

            pending = None  # (tc0, av tiles) awaiting postprocess
            for tc0 in range(0, T, TCH):
                av = [pav_pool.tile([P, 2 * H * E1], F32,
                                    name=f"av{i}", tag=f"av{i}")
                      for i in range(2)]
                for si in range(NS):
                    s0 = si * P
                    ptiles = []
                    for h in range(H):
                        kT_t, qT_t, pb = hsrc(h)
                        stp = pst_pool.tile([P, TCH], F32,
                                            name="stp", tag="stp")
                        nc.tensor.matmul(
                            stp, kT_t[pb:pb + HS, s0:s0 + P],
                            qT_t[pb:pb + HS, tc0:tc0 + TCH],
                            start=True, stop=True, tile_position=(pb, 0))
                        ptp = pt_pool.tile([P, TCH], BF16,
                                           name="ptp", tag="ptp")
                        if exp_pat(si, h) == "a":
                            nc.scalar.activation(ptp, stp, Exp, scale=SCALE)
                        else:
                            nc.vector.tensor_scalar(
                                ptp.bitcast(I16), stp, S1, S2,
                                Alu.mult, Alu.add)
                        ptiles.append(ptp)
                    if si == 0 and pending is not None:
                        postprocess(*pending)
                        pending = None
                    for h in range(H):
                        for tt in range(NT):
                            av_t = av[tt // 2].rearrange(
                                "p (u h e) -> p u h e", u=2, h=H)
                            nc.tensor.matmul(
                                av_t[:, tt % 2, h, :],
                                ptiles[h][:, tt * P:(tt + 1) * P],
                                va_r[:, si, h, :],
                                start=(si == 0 and h == 0 and tt % 2 == 0),
                                stop=(si == NS - 1),
                                skip_group_check=True)
                pending = (tc0, av)
            postprocess(*pending)

    nc.compile()
    return nc


def _get_nc():
    if "nc" not in _CACHE:
        _CACHE["nc"] = build_nc()
    return _CACHE["nc"]


def make_in_maps(x, Wq, Wk, Wv, Wproj, bproj):
    bf = ml_dtypes.bfloat16
    x = np.asarray(x, np.float32)
    pack = lambda w: np.ascontiguousarray(
        np.transpose(np.asarray(w, np.float32), (1, 0, 2)).reshape(C, H * HS)
    ).astype(bf)
    wq, wk, wv = pack(Wq), pack(Wk), pack(Wv)
    wp = np.ascontiguousarray(
        np.asarray(Wproj, np.float32).reshape(H * HS, C)).astype(bf)
    bp = np.asarray(bproj, np.float32).reshape(1, C).astype(bf)
    ident = np.eye(P, dtype=np.float32).astype(bf)
    maps = []
    for i in range(B):
        xti = np.ascontiguousarray(x[i].T).astype(bf)
        maps.append({"xT": xti, "wq": wq, "wk": wk, "wv": wv,
                     "wp": wp, "bp": bp, "ident": ident})
    return maps


def run(inputs, trace=False, **kw):
    nc = _get_nc()
    in_maps = make_in_maps(**inputs)
    res = run_bass_kernel_spmd(nc, in_maps, core_ids=list(range(B)),
                               trace=trace, **kw)
    y = np.stack([np.asarray(res.results[i]["out"], np.float32)
                  for i in range(B)], axis=0)
    return y, res


def kernel(**inputs):
    y, _ = run(inputs, trace=False)
    return y


# revision 7
# speedup vs baseline: 1.4277x; 1.1015x over previous
"""Multi-head self-attention (B=8, T=2048, C=192, H=6, HS=32) on 8 TRN2 cores.

Sharding: data-parallel over batch — core i computes batch element i fully
on-chip (no collectives). Host pre-transposes x and packs weights so the
device does zero transposes.

Per-core pipeline (engine in parentheses):
  qT/kT [d, t] = W^T @ xT                   (PE; PSUM->SBUF copies on ACT)
  v_aug [s, (h|1)]                          (PE; DVE copies; ones col -> rowsum)
  S^T   [s, t] = kT_h^T @ qT_h              (PE, K=32, one f32 PSUM bank/head)
  P^T = exp(S/sqrt(HS)) per head tile:      exact exp on ACT, or Schraudolph
        bf16-bit affine on DVE (bits = trunc(S1*x + S2) as int16, bitcast
        to bf16)
  O[t, d|rowsum] += P^T_tile^T @ v_aug      (PE, free dim 33 per head)
  O' = O * recip(rowsum)                    (DVE recip + broadcast mul)
  O'^T via PE transpose (identity), DVE copies to SBUF
  y[t, c] = O'^T.T @ Wp + bias              (PE, K=192 in 2 chunks + bias row)
"""

import numpy as np
import ml_dtypes
from contextlib import ExitStack

import concourse.bass as bass
import concourse.tile as tile
from concourse import bacc, mybir
from concourse.bass_utils import run_bass_kernel_spmd

B, T, C = 8, 2048, 192
H, HS = 6, 32
P = 128
TCH = 512            # t-chunk width per head (pair tile = 2*TCH)
NT = T // TCH        # 4
NS = T // P          # 16 s-tiles
E1 = HS + 1          # 33: per-head AV free dim (32 d + rowsum)
SCALE = 1.0 / float(np.sqrt(HS))
BF16 = mybir.dt.bfloat16
F32 = mybir.dt.float32
I16 = mybir.dt.int16
Exp = mybir.ActivationFunctionType.Exp
Alu = mybir.AluOpType

# Schraudolph bf16-bits exp: bf16(trunc(S1*x + S2)) ~ exp(SCALE*x), |rel|<4%
S1 = float((128.0 / np.log(2.0)) * SCALE)
S2 = 16250.0

# exp engine per (si, head): 'a' = ACT exact exp, 'd' = DVE Schraudolph.
# Heads 0-2 exact on ACT (plus head 3 every 4th si); heads 3-5 approx on DVE.
def exp_pat(si, h):
    if h < 3:
        return "a"
    if h == 3 and si % 4 == 0:
        return "a"
    return "d"

_CACHE = {}


def build_nc():
    nc = bacc.Bacc()
    xT = nc.declare_dram_parameter("xT", [C, T], BF16, isOutput=False)
    wq = nc.declare_dram_parameter("wq", [C, H * HS], BF16, isOutput=False)
    wk = nc.declare_dram_parameter("wk", [C, H * HS], BF16, isOutput=False)
    wv = nc.declare_dram_parameter("wv", [C, H * HS], BF16, isOutput=False)
    wp = nc.declare_dram_parameter("wp", [H * HS, C], BF16, isOutput=False)
    bp = nc.declare_dram_parameter("bp", [1, C], BF16, isOutput=False)
    ident = nc.declare_dram_parameter("ident", [P, P], BF16, isOutput=False)
    out = nc.declare_dram_parameter("out", [T, C], F32, isOutput=True)

    with tile.TileContext(nc) as tc, ExitStack() as ctx:
        singles = ctx.enter_context(tc.tile_pool(name="singles", bufs=1))
        qk_pool = ctx.enter_context(tc.tile_pool(name="qk", bufs=1))
        pt_pool = ctx.enter_context(tc.tile_pool(name="ptp", bufs=10))
        post_pool = ctx.enter_context(tc.tile_pool(name="post", bufs=2))
        ysb_pool = ctx.enter_context(tc.tile_pool(name="ysb", bufs=3))

        # ---------------- load inputs ----------------
        xT_a = singles.tile([P, T], BF16)
        nc.sync.dma_start(xT_a, xT[0:P, :])
        xT_b = singles.tile([C - P, T], BF16)
        nc.sync.dma_start(xT_b, xT[P:C, :])

        w_sb = {}
        for name, dram in (("q", wq), ("k", wk), ("v", wv)):
            a = singles.tile([P, H * HS], BF16, name=f"w{name}a")
            nc.sync.dma_start(a, dram[0:P, :])
            b = singles.tile([C - P, H * HS], BF16, name=f"w{name}b")
            nc.sync.dma_start(b, dram[P:C, :])
            w_sb[name] = (a, b)

        wp_a = singles.tile([P, C], BF16, name="wpa")
        nc.sync.dma_start(wp_a, wp[0:P, :])
        wp_b = singles.tile([H * HS - P, C], BF16, name="wpb")
        nc.sync.dma_start(wp_b, wp[P:H * HS, :])
        bp_sb = singles.tile([1, C], BF16)
        nc.sync.dma_start(bp_sb, bp[:, :])
        id_sb = singles.tile([P, P], BF16, name="idsb")
        nc.sync.dma_start(id_sb, ident[:, :])
        ones1 = singles.tile([1, P], BF16)
        nc.gpsimd.memset(ones1, 1.0)

        # preload exp activation table off the critical path
        warm = singles.tile([1, P], BF16, name="warm")
        nc.scalar.activation(warm, ones1, Exp)

        # v_aug: [s, si*(h|1)] with ones in col 32 of each head group
        v_aug = singles.tile([P, NS * H * E1], BF16, name="vaug")
        nc.gpsimd.memset(v_aug, 1.0)
        va_r = v_aug.rearrange("p (s h e) -> p s h e", s=NS, h=H)

        # ---------------- phase 1: qT, kT, v_aug ----------------
        qT_a = qk_pool.tile([P, T], BF16)       # heads 0..3, d-major
        qT_b = qk_pool.tile([C - P, T], BF16)   # heads 4,5
        kT_a = qk_pool.tile([P, T], BF16)
        kT_b = qk_pool.tile([C - P, T], BF16)
        with tc.tile_pool(name="pqkv", bufs=2, space="PSUM") as pqkv:
            for proj, dst_a, dst_b in (("q", qT_a, qT_b), ("k", kT_a, kT_b)):
                wa, wb = w_sb[proj]
                for dlo, dsz, dst in ((0, P, dst_a), (P, C - P, dst_b)):
                    for t0 in range(0, T, 2 * TCH):
                        ps = pqkv.tile([P, 2 * TCH], F32, name="psq", tag="psq")
                        for th in (0, TCH):
                            nc.tensor.matmul(
                                ps[0:dsz, th:th + TCH], wa[:, dlo:dlo + dsz],
                                xT_a[:, t0 + th:t0 + th + TCH],
                                start=True, stop=False)
                            nc.tensor.matmul(
                                ps[0:dsz, th:th + TCH], wb[:, dlo:dlo + dsz],
                                xT_b[:, t0 + th:t0 + th + TCH],
                                start=False, stop=True)
                        nc.scalar.copy(
                            dst[0:dsz, t0:t0 + 2 * TCH], ps[0:dsz, :])
            wva, wvb = w_sb["v"]
            for si in range(NS):
                s0 = si * P
                ps = pqkv.tile([P, H * HS], F32, name="psv", tag="psv")
                nc.tensor.matmul(ps, xT_a[:, s0:s0 + P], wva,
                                 start=True, stop=False)
                nc.tensor.matmul(ps, xT_b[:, s0:s0 + P], wvb,
                                 start=False, stop=True)
                ps_r = ps.rearrange("p (h d) -> p h d", h=H)
                nc.vector.tensor_copy(va_r[:, si, :, 0:HS], ps_r)

        # ---------------- phase 2: attention ----------------
        def hsrc(h):
            if h < 4:
                return kT_a, qT_a, HS * h
            return kT_b, qT_b, HS * (h - 4)

        with (
            tc.tile_pool(name="pstA", bufs=2, space="PSUM") as pstA_pool,
            tc.tile_pool(name="pstD", bufs=2, space="PSUM") as pstD_pool,
            tc.tile_pool(name="pav", bufs=1, space="PSUM") as pav_pool,
            tc.tile_pool(name="py", bufs=2, space="PSUM") as py_pool,
        ):
            def postprocess(tc0, av, tt):
                """normalize, transpose, project, store one t-tile."""
                if True:
                    av_t = av[tt // 2].rearrange(
                        "p (u h e) -> p u h e", u=2, h=H)
                    u = tt % 2
                    rec = post_pool.tile([P, H], F32, name="rec", tag="rec")
                    nc.vector.reciprocal(rec, av_t[:, u, :, HS])
                    onorm = post_pool.tile([P, H * HS], BF16,
                                           name="onorm", tag="onorm")
                    on_r = onorm.rearrange("p (h e) -> p h e", h=H)
                    nc.vector.tensor_tensor(
                        on_r, av_t[:, u, :, 0:HS],
                        rec.unsqueeze(2).to_broadcast([P, H, HS]),
                        Alu.mult)
                    # proj psum bank doubles as transpose scratch: cols
                    # [C, C+P) viewed as bf16 hold O'^T before DVE copy-out
                    ps = py_pool.tile([P, C + P], F32, name="psy", tag="psy")
                    tp = ps[:, C:C + P].bitcast(BF16)
                    nc.tensor.transpose(tp[:, 0:P], onorm[:, 0:P], id_sb)
                    nc.tensor.transpose(
                        tp[0:H * HS - P, P:2 * P], onorm[:, P:H * HS], id_sb)
                    oT1 = post_pool.tile([P, P], BF16, name="oT1", tag="oT1")
                    nc.vector.tensor_copy(oT1, tp[:, 0:P])
                    oT2 = post_pool.tile([H * HS - P, P], BF16,
                                         name="oT2", tag="oT2")
                    nc.vector.tensor_copy(oT2, tp[0:H * HS - P, P:2 * P])
                    nc.tensor.matmul(ps[:, 0:C], ones1, bp_sb,
                                     start=True, stop=False)
                    nc.tensor.matmul(ps[:, 0:C], oT1, wp_a,
                                     start=False, stop=False)
                    nc.tensor.matmul(ps[:, 0:C], oT2, wp_b,
                                     start=False, stop=True)
                    ysb = ysb_pool.tile([P, C], F32, name="ysbt", tag="ysbt")
                    nc.vector.tensor_copy(ysb, ps[:, 0:C])
                    nc.sync.dma_start(out[tc0 + tt * P:tc0 + (tt + 1) * P, :],
                                      ysb)

            pending = None  # (tc0, av tiles) awaiting postprocess
            for tc0 in range(0, T, TCH):
                av = [pav_pool.tile([P, 2 * H * E1], F32,
                                    name=f"av{i}", tag=f"av{i}")
                      for i in range(2)]
                for si in range(NS):
                    s0 = si * P
                    ptiles = []
                    for h in range(H):
                        kT_t, qT_t, pb = hsrc(h)
                        is_act = exp_pat(si, h) == "a"
                        pool, tag = ((pstA_pool, "stA") if is_act
                                     else (pstD_pool, "stD"))
                        stp = pool.tile([P, TCH], F32, name="stp", tag=tag)
                        nc.tensor.matmul(
                            stp, kT_t[pb:pb + HS, s0:s0 + P],
                            qT_t[pb:pb + HS, tc0:tc0 + TCH],
                            start=True, stop=True, tile_position=(pb, 0))
                        ptp = pt_pool.tile([P, TCH], BF16,
                                           name="ptp", tag="ptp")
                        if is_act:
                            nc.scalar.activation(ptp, stp, Exp, scale=SCALE)
                        else:
                            nc.vector.tensor_scalar(
                                ptp.bitcast(I16), stp, S1, S2,
                                Alu.mult, Alu.add)
                        ptiles.append(ptp)
                    if si < NT and pending is not None:
                        postprocess(*pending, tt=si)
                        if si == NT - 1:
                            pending = None
                    for h in range(H):
                        for tt in range(NT):
                            av_t = av[tt // 2].rearrange(
                                "p (u h e) -> p u h e", u=2, h=H)
                            nc.tensor.matmul(
                                av_t[:, tt % 2, h, :],
                                ptiles[h][:, tt * P:(tt + 1) * P],
                                va_r[:, si, h, :],
                                start=(si == 0 and h == 0 and tt % 2 == 0),
                                stop=(si == NS - 1),
                                skip_group_check=True)
                pending = (tc0, av)
            for tt in range(NT):
                postprocess(*pending, tt=tt)

    nc.compile()
    return nc


def _get_nc():
    if "nc" not in _CACHE:
        _CACHE["nc"] = build_nc()
    return _CACHE["nc"]


def make_in_maps(x, Wq, Wk, Wv, Wproj, bproj):
    bf = ml_dtypes.bfloat16
    x = np.asarray(x, np.float32)
    pack = lambda w: np.ascontiguousarray(
        np.transpose(np.asarray(w, np.float32), (1, 0, 2)).reshape(C, H * HS)
    ).astype(bf)
    wq, wk, wv = pack(Wq), pack(Wk), pack(Wv)
    wp = np.ascontiguousarray(
        np.asarray(Wproj, np.float32).reshape(H * HS, C)).astype(bf)
    bp = np.asarray(bproj, np.float32).reshape(1, C).astype(bf)
    ident = np.eye(P, dtype=np.float32).astype(bf)
    maps = []
    for i in range(B):
        xti = np.ascontiguousarray(x[i].T).astype(bf)
        maps.append({"xT": xti, "wq": wq, "wk": wk, "wv": wv,
                     "wp": wp, "bp": bp, "ident": ident})
    return maps


def run(inputs, trace=False, **kw):
    nc = _get_nc()
    in_maps = make_in_maps(**inputs)
    res = run_bass_kernel_spmd(nc, in_maps, core_ids=list(range(B)),
                               trace=trace, **kw)
    y = np.stack([np.asarray(res.results[i]["out"], np.float32)
                  for i in range(B)], axis=0)
    return y, res


def kernel(**inputs):
    y, _ = run(inputs, trace=False)
    return y
